# revision 1
# baseline (speedup 1.0000x reference)
"""Causal multi-head self-attention with RoPE on 8 Trainium2 NeuronCores.

Sharding: core = (batch b, head-group g) with b = core//2, g = core%2.
Each core computes QKV projections for its batch element restricted to its
8 heads (512 of 1024 projection rows), RoPE, causal attention, and the
partial output projection y_g = attn_g @ wo[:, g*512:(g+1)*512].T.  The host
sums the two head-group partials per batch element.

v2 redesign (vs the 498us/421us baseline):
- Phase 2 was latency-bound: program order sc_k, pv_k, sc_{k+1} stalled the
  in-order PE on every exp (measured 1082ns/iter vs ~500ns of work).  Scores
  and PV are now software-pipelined (pv_k issues after sc_{k+1}).
- The softmax exp is the hard floor (ACT engine is the only exp engine,
  ~17.4M elements/core ~ 116us).  Everything else is arranged to hide under
  it: probs+V in fp8e4 with DoubleRow matmuls (2 key-blocks per instruction),
  output projection in fp8e4 DoubleRow, bf16 chained PE transposes (1 bank,
  one DVE eviction), causal mask applied by an extra PE matmul (triT @ I)
  instead of a DVE add, exp shifted by -3 so fp8 probs can't overflow
  (e4m3 max 240; unshifted scores reach e^5.5).
- The per-q-group normalize + output projection is deferred and woven into
  the next q-group's head loop so the ACT engine never drains.

Scores stay bf16 (fp8 q/k would put ~8% noise on scores pre-softmax).
V is computed in bf16 and quantized to fp8 on eviction.
"""
import math
import os
from contextlib import ExitStack

import numpy as np
import ml_dtypes

import concourse.bass as bass
import concourse.tile as tile
from concourse import bacc, mybir
from concourse import masks
from concourse.bass_utils import run_bass_kernel_spmd

F32 = mybir.dt.float32
BF16 = mybir.dt.bfloat16
FP8 = mybir.dt.float8e4
U8 = mybir.dt.uint8
DRM = mybir.MatmulPerfMode.DoubleRow
EXPF = mybir.ActivationFunctionType.Exp

D = 1024          # d_model
NH = 16           # heads total
DK = 64           # head dim
S = 2048          # sequence
B = 4             # batch
THETA = 10000.0
HPG = 8           # heads per group (2 groups over 8 cores with 4 batches)
W = HPG * DK      # 512: local projection width
NSB = S // 128    # 16 s-blocks
NQG = 4           # 512-wide q groups
NEG = -1.0e10     # additive causal mask value
ESH = -3.0        # exp shift: exp(s/8 - 3); cancels in normalization
SCL = 1.0 / math.sqrt(DK)

# fp8 paths exist but are OFF by default: e4m3's 3-bit mantissa puts
# ~3.5e-2 max-norm noise on the output (measured in sim AND in a pure
# numpy model), over the 2e-2 budget.  bf16 keeps all the structural wins.
KPV = os.environ.get("KPV", "bf16")     # probs @ v matmul: "fp8" | "bf16"
KOP = os.environ.get("KOP", "bf16")     # output projection: "fp8" | "bf16"
MODE = f"v2-bf16 (pv={KPV}, oproj={KOP})"
TRACE = bool(int(os.environ.get("KTRACE", "0")))

_cache = {}


def build_nc():
    vdt = FP8 if KPV == "fp8" else BF16    # vt + ex (probs) dtype
    vdt2 = vdt
    odt = FP8 if KOP == "fp8" else BF16    # wo + aq dtype
    nc = bacc.Bacc(None, target_bir_lowering=False, debug=False)

    xt = nc.dram_tensor("xt", [D, S], BF16, kind="ExternalInput")
    wqt = nc.dram_tensor("wqt", [D, W], BF16, kind="ExternalInput")
    wkt = nc.dram_tensor("wkt", [D, W], BF16, kind="ExternalInput")
    wvt = nc.dram_tensor("wvt", [D, W], BF16, kind="ExternalInput")
    wot = nc.dram_tensor("wot", [W, D], odt, kind="ExternalInput")
    cosb = nc.dram_tensor("cosb", [S, W // 2], F32, kind="ExternalInput")
    sinb = nc.dram_tensor("sinb", [S, W // 2], F32, kind="ExternalInput")
    yp = nc.dram_tensor("yp", [S, D], F32, kind="ExternalOutput")

    xt3 = xt[:].rearrange("(jo p) s -> p jo s", p=128)       # [128, 8, S]
    wqt3 = wqt[:].rearrange("(jo p) i -> p jo i", p=128)     # [128, 8, W]
    wkt3 = wkt[:].rearrange("(jo p) i -> p jo i", p=128)
    wvt3 = wvt[:].rearrange("(jo p) i -> p jo i", p=128)
    wot3 = wot[:].rearrange("(jo p) i -> p jo i", p=128)     # [128, 4, D]

    with tile.TileContext(nc, pool_alloc_mode="queue") as tc, \
            ExitStack() as ctx:
        persist = ctx.enter_context(tc.tile_pool(name="persist", bufs=1))
        identb = persist.tile([128, 128], BF16, name="identb")
        masks.make_identity(nc, identb)
        bsh = persist.tile([128, 1], F32, name="bsh")
        nc.gpsimd.memset(bsh, ESH)

        # persistent activations: q^T / k^T as [128, slab, S] (slab j holds
        # the 128 dims of heads (2j, 2j+1)); v s-major in s-block PAIRS with
        # a ones column per head for the softmax denominator.
        qT4 = persist.tile([128, 4, S], BF16, name="qT4")
        kT4 = persist.tile([128, 4, S], BF16, name="kT4")
        vtp = [persist.tile([128, 2, HPG, DK + 1], vdt, name=f"vtp{i}")
               for i in range(NSB // 2)]

        # ---------------- phase 1: projections + RoPE + transposes --------
        with tc.tile_pool(name="wp", bufs=1) as wp, \
             tc.tile_pool(name="p1t", bufs=3) as p1t, \
             tc.tile_pool(name="p1p", bufs=3, space="PSUM") as p1p, \
             tc.tile_pool(name="p1tr", bufs=2, space="PSUM") as p1tr:
            # wq loaded as 8 separate per-chunk tiles on the (idle) scalar
            # queue so the first projection matmul waits on a 128KB DMA, not
            # the full 1MB, and the sync queue starts with the x stream
            wq_j = [wp.tile([128, W], BF16, name=f"wq_j{jo}")
                    for jo in range(8)]
            for jo in range(8):
                nc.scalar.dma_start(wq_j[jo][:], wqt3[:, jo, :])
            wk_s = wp.tile([128, 8, W], BF16, name="wk_s")
            wv_s = wp.tile([128, 8, W], BF16, name="wv_s")
            # k/v weight loads issued on a different queue so the first
            # projection matmul only waits for wq + the first x slice
            nc.gpsimd.dma_start(wk_s[:], wkt3[:])
            nc.gpsimd.dma_start(wv_s[:], wvt3[:])

            def rope(ps, outt, c3, s3):
                # ps: [128, W] PSUM (pre-RoPE proj, s-major, heads as
                # [evens(32) | odds(32)] blocks); outt: [128, W] SBUF bf16
                pe = ps.rearrange("p (h eo c) -> p h eo c", eo=2, c=32)
                ein, oin = pe[:, :, 0, :], pe[:, :, 1, :]
                oe = outt.rearrange("p (h eo c) -> p h eo c", eo=2, c=32)
                eout, oout = oe[:, :, 0, :], oe[:, :, 1, :]
                ra = p1t.tile([128, 8, 32], F32, name="ra", tag="ra")
                rb = p1t.tile([128, 8, 32], F32, name="rb", tag="rb")
                nc.vector.tensor_mul(ra, ein, c3)
                nc.vector.tensor_mul(rb, oin, s3)
                nc.vector.tensor_sub(eout, ra, rb)
                rc = p1t.tile([128, 8, 32], F32, name="rc", tag="rc")
                rd = p1t.tile([128, 8, 32], F32, name="rd", tag="rd")
                nc.vector.tensor_mul(rc, ein, s3)
                nc.vector.tensor_mul(rd, oin, c3)
                nc.vector.tensor_add(oout, rc, rd)

            def emit_transposes(q_ro, k_ro, s0):
                # 8 chained bf16 transposes into one PSUM bank, then two
                # strided DVE evictions into the d-major slabs
                ptr8 = p1tr.tile([128, 8, 128], BF16, name="ptr8", tag="tr")
                for pr in range(4):
                    nc.tensor.matmul(
                        ptr8[:, pr, :], q_ro[:, pr * 128:(pr + 1) * 128],
                        identb[:], is_transpose=True,
                        start=(pr == 0), stop=False)
                for pr in range(4):
                    nc.tensor.matmul(
                        ptr8[:, 4 + pr, :], k_ro[:, pr * 128:(pr + 1) * 128],
                        identb[:], is_transpose=True,
                        start=False, stop=(pr == 3))
                nc.vector.tensor_copy(qT4[:, :, s0:s0 + 128], ptr8[:, 0:4, :])
                nc.vector.tensor_copy(kT4[:, :, s0:s0 + 128], ptr8[:, 4:8, :])

            pending = None
            for sb in range(NSB):
                s0 = sb * 128
                xs = p1t.tile([128, 8, 128], BF16, name="xs", tag="xs")
                nc.sync.dma_start(xs[:], xt3[:, :, s0:s0 + 128])
                cs = p1t.tile([128, W // 2], F32, name="cs", tag="cs")
                nc.sync.dma_start(cs[:], cosb[s0:s0 + 128, :])
                sn = p1t.tile([128, W // 2], F32, name="sn", tag="sn")
                nc.sync.dma_start(sn[:], sinb[s0:s0 + 128, :])
                c3 = cs.rearrange("p (h c) -> p h c", c=32)
                s3 = sn.rearrange("p (h c) -> p h c", c=32)

                pq = p1p.tile([128, W], F32, name="pq", tag="pp")
                pk = p1p.tile([128, W], F32, name="pk", tag="pp")
                pv = p1p.tile([128, W], F32, name="pv", tag="pp")
                for jo in range(8):
                    nc.tensor.matmul(pq[:], xs[:, jo, :], wq_j[jo][:],
                                     start=(jo == 0), stop=(jo == 7))
                for dst, wsb in ((pk, wk_s), (pv, wv_s)):
                    for jo in range(8):
                        nc.tensor.matmul(
                            dst[:], xs[:, jo, :],
                            wsb[:, jo, :],
                            start=(jo == 0), stop=(jo == 7))

                q_ro = p1t.tile([128, W], BF16, name="q_ro", tag="qro",
                                bufs=2)
                rope(pq, q_ro, c3, s3)
                k_ro = p1t.tile([128, W], BF16, name="k_ro", tag="kro",
                                bufs=2)
                rope(pk, k_ro, c3, s3)

                # v eviction (+ ones column per head) on DVE — the ACT
                # engine is the phase-2 bottleneck, keep it fully free
                v3 = vtp[sb // 2][:, sb % 2, :, :]       # [128, 8, 65]
                nc.vector.tensor_copy(v3[:, :, 0:DK],
                                      pv.rearrange("p (h c) -> p h c", c=DK))
                ones_bits = 0x38 if vdt == FP8 else 0x3F80  # 1.0
                odt_cast = U8 if vdt == FP8 else mybir.dt.uint16
                nc.gpsimd.memset(v3[:, :, DK:DK + 1].bitcast(odt_cast),
                                 ones_bits)

                # transposes of the previous s-block (software pipeline so
                # the PE doesn't stall on this block's RoPE)
                if pending is not None:
                    emit_transposes(*pending)
                pending = (q_ro, k_ro, s0)
            emit_transposes(*pending)

        # ------------- phase 2+3: attention + output projection -----------
        with tc.tile_pool(name="p2c", bufs=1) as p2c, \
             tc.tile_pool(name="p2t", bufs=3) as p2t, \
             tc.tile_pool(name="aqp", bufs=2) as aqp, \
             tc.tile_pool(name="exp", bufs=6) as expp, \
             tc.tile_pool(name="scp", bufs=4, space="PSUM") as scp, \
             tc.tile_pool(name="pvp", bufs=2, space="PSUM") as pvp, \
             tc.tile_pool(name="pyp", bufs=2, space="PSUM") as pyp:
            wo_s = p2c.tile([128, 4, D], odt, name="wo_s")
            nc.sync.dma_start(wo_s[:], wot3[:])

            def attn_head(qg, h, stage8):
                """Scores + exp + PV for one head, software-pipelined:
                pv_i is emitted after the scores of unit i+1."""
                slab, r0 = h // 2, 64 * (h % 2)
                q0 = qg * 512
                pvh = pvp.tile([DK + 1, 512], F32, name="pvh", tag="pv",
                               bufs=2)
                emitters = []

                def drive(last=False):
                    # emit the pv of units two behind the one just added
                    # (or everything left, at the end of the head) so the
                    # exp -> (mask) -> pv latency is hidden by other PE work
                    n = len(emitters)
                    lo = drive.done
                    hi = n if last else max(n - 2, 0)
                    for i in range(lo, hi):
                        emitters[i](i == 0, last and i == n - 1)
                    drive.done = hi
                drive.done = 0

                nkb = 4 * qg + 4
                for kb in range(nkb):        # one unit per 128-key block
                    off = kb - 4 * qg
                    c0 = 128 * max(off, 0)
                    wd = 512 - c0
                    sc = scp.tile([128, 512], F32, name="sc", tag="sc")
                    nc.tensor.matmul(
                        sc[:, c0:512],
                        kT4[r0:r0 + DK, slab, kb * 128:(kb + 1) * 128],
                        qT4[r0:r0 + DK, slab, q0 + c0:q0 + 512],
                        start=True, stop=True)
                    ex = expp.tile([128, 512], vdt2, name="ex", tag="ex")
                    nc.scalar.activation(ex[:, 0:wd], sc[:, c0:512],
                                         EXPF, scale=SCL, bias=bsh[:, 0:1])
                    if off >= 0:
                        # causal mask for the 128-wide diagonal square: zero
                        # the q < k half of the exp'd probs on the (idle)
                        # gpsimd.  A PE mask matmul costs ~216ns regardless
                        # of width (30us total); here it's hidden.
                        nc.gpsimd.affine_select(
                            out=ex[:, 0:128], in_=ex[:, 0:128],
                            compare_op=mybir.AluOpType.is_ge, fill=0.0,
                            base=0, pattern=[[1, 128]], channel_multiplier=-1)
                    emitters.append(
                        lambda st, sp, kb=kb, c0=c0, wd=wd, ex=ex:
                        nc.tensor.matmul(
                            pvh[:, c0:512],
                            vtp[kb // 2][:, kb % 2, h, :],
                            ex[:, 0:wd], start=st, stop=sp))
                    drive()
                drive(last=True)

                # evict unnormalized attn^T rows; stage the denominator row
                # (batched reciprocal happens once per q-group)
                nc.vector.tensor_copy(aqb_cur[0][r0:r0 + 64, slab, :],
                                      pvh[0:DK, :])
                dsb = p2t.tile([1, 512], F32, name="dsb", tag="dsb",
                               bufs=3)
                nc.vector.tensor_copy(dsb[:], pvh[DK:DK + 1, :])
                nc.sync.dma_start(stage8[h:h + 1, :], dsb[:])

            def make_deferred(qg, stage8, aqb, aq8):
                """Normalize + output projection for q-group qg, split into
                9 steps that are woven into the next q-group's head loop."""
                steps = []
                rall = [None]

                def s_recip():
                    r = p2t.tile([HPG, 512], F32, name="rall8", tag="r8",
                                 bufs=2)
                    nc.vector.reciprocal(r[:], stage8[:])
                    rall[0] = r
                steps.append(s_recip)

                def norm_pair(hp):
                    for h in (2 * hp, 2 * hp + 1):
                        slab, r0 = h // 2, 64 * (h % 2)
                        rsb = p2t.tile([1, 512], F32, name="rsb", tag="rsb",
                                       bufs=2)
                        nc.sync.dma_start(rsb[:], rall[0][h:h + 1, :])
                        rbc = p2t.tile([128, 512], F32, name="rbc",
                                       tag="rbc", bufs=2)
                        nc.gpsimd.partition_broadcast(rbc[:], rsb[:],
                                                      channels=128)
                        nc.vector.tensor_mul(aq8[r0:r0 + 64, slab, :],
                                             aqb[r0:r0 + 64, slab, :],
                                             rbc[r0:r0 + 64, :])
                for hp in range(4):
                    steps.append(lambda hp=hp: norm_pair(hp))

                def oproj(sbl):
                    s0 = qg * 512 + sbl * 128
                    yt = p2t.tile([128, 1024], F32, name="yt", tag="yt",
                                  bufs=2)
                    for ih in range(2):
                        # 1-bank py double-buffered: the eviction of half ih
                        # overlaps the matmuls of the next half
                        py = pyp.tile([128, 512], F32, name="py", tag="py",
                                      bufs=2)
                        if KOP == "fp8":
                            for jp in range(2):
                                nc.tensor.matmul(
                                    py[:],
                                    aq8[:, 2 * jp:2 * jp + 2,
                                        sbl * 128:(sbl + 1) * 128],
                                    wo_s[:, 2 * jp:2 * jp + 2,
                                         ih * 512:(ih + 1) * 512],
                                    start=(jp == 0), stop=(jp == 1),
                                    perf_mode=DRM)
                        else:
                            for j in range(4):
                                nc.tensor.matmul(
                                    py[:],
                                    aq8[:, j, sbl * 128:(sbl + 1) * 128],
                                    wo_s[:, j, ih * 512:(ih + 1) * 512],
                                    start=(j == 0), stop=(j == 3))
                        nc.vector.tensor_copy(
                            yt[:, ih * 512:(ih + 1) * 512], py[:])
                    nc.sync.dma_start(yp[s0:s0 + 128, :], yt[:])
                for sbl in range(4):
                    steps.append(lambda sbl=sbl: oproj(sbl))
                return steps

            deferred = []
            for qg in range(NQG):
                stage8 = p2t.tile([HPG, 512], F32, name="stage8",
                                  tag="stage8", bufs=2)
                aqb = aqp.tile([128, 4, 512], BF16, name="aqb", tag="aqb",
                               bufs=2)
                aq8 = aqp.tile([128, 4, 512], FP8, name="aq8", tag="aq8",
                               bufs=2) if KOP == "fp8" else aqb
                aqb_cur = (aqb,)
                for h in range(HPG):
                    attn_head(qg, h, stage8)
                    if h < len(deferred):
                        deferred[h]()
                for st in deferred[HPG:]:
                    st()
                deferred = make_deferred(qg, stage8, aqb, aq8)
            for st in deferred:
                st()

    nc.compile()
    return nc


def _prep_inputs(x, token_positions, wq, wk, wv, wo):
    bf16 = ml_dtypes.bfloat16
    odt = ml_dtypes.float8_e4m3 if KOP == "fp8" else bf16
    # per-head permutation: [0,2,...,62, 1,3,...,63] (evens then odds)
    pi = np.concatenate([np.arange(0, DK, 2), np.arange(1, DK, 2)])
    perm = (np.arange(NH)[:, None] * DK + pi[None, :]).reshape(-1)
    wq_p = wq[perm, :]
    wk_p = wk[perm, :]

    pos = np.asarray(token_positions).astype(np.float32)
    thetas = (1.0 / (THETA ** (2.0 * np.arange(DK // 2, dtype=np.float32)
                               / DK))).astype(np.float32)
    ang = np.outer(pos, thetas).astype(np.float32)          # [S, 32]
    cos = np.tile(np.cos(ang), (1, HPG)).astype(np.float32)  # [S, 256]
    sin = np.tile(np.sin(ang), (1, HPG)).astype(np.float32)

    in_maps = []
    for core in range(8):
        b, g = core // 2, core % 2
        gs = slice(g * W, (g + 1) * W)
        in_maps.append({
            "xt": np.ascontiguousarray(x[b].T).astype(bf16),
            "wqt": np.ascontiguousarray(wq_p[gs, :].T).astype(bf16),
            "wkt": np.ascontiguousarray(wk_p[gs, :].T).astype(bf16),
            "wvt": np.ascontiguousarray(wv[gs, :].T).astype(bf16),
            "wot": np.ascontiguousarray(wo[:, gs].T).astype(odt),
            "cosb": cos,
            "sinb": sin,
        })
    return in_maps


last_exec_time_ns = None


def _install_ntff_hook_shim():
    """This image's antenv lacks axon_hooks; wire the ctypes NTFF hook from
    trn_agent_boot so trace=True yields HW exec times."""
    import sys as _sys
    import types as _types
    try:
        from antenv import axon_hooks  # noqa: F401
        return
    except ImportError:
        pass
    from trn_agent_boot.trn_boot import _ntff_profile_via_ctypes
    hook = _ntff_profile_via_ctypes("/opt/axon/libaxon_pjrt.so")
    mod = _types.ModuleType("antenv.axon_hooks")
    mod.get_axon_ntff_profile_hook = lambda: hook
    _sys.modules["antenv.axon_hooks"] = mod


def kernel(x, token_positions, wq, wk, wv, wo):
    global last_exec_time_ns
    x = np.asarray(x, dtype=np.float32)
    token_positions = np.asarray(token_positions)
    wq = np.asarray(wq, dtype=np.float32)
    wk = np.asarray(wk, dtype=np.float32)
    wv = np.asarray(wv, dtype=np.float32)
    wo = np.asarray(wo, dtype=np.float32)

    if "nc" not in _cache:
        _cache["nc"] = build_nc()
    nc = _cache["nc"]

    in_maps = _prep_inputs(x, token_positions, wq, wk, wv, wo)
    res = None
    if TRACE:
        try:
            _install_ntff_hook_shim()
            res = run_bass_kernel_spmd(nc, in_maps, list(range(8)),
                                       trace=True,
                                       trace_cores=list(range(8)))
        except Exception as e:  # profiling must never sink correctness
            print(f"trace run failed ({type(e).__name__}: {e}); "
                  f"retrying untraced")
            res = None
    if res is None:
        res = run_bass_kernel_spmd(nc, in_maps, list(range(8)))
    last_exec_time_ns = res.exec_time_ns

    out = np.empty((B, S, D), dtype=np.float32)
    for b in range(B):
        out[b] = res.results[2 * b]["yp"] + res.results[2 * b + 1]["yp"]
    return out



# revision 2
# speedup vs baseline: 1.0766x; 1.0766x over previous
"""Causal multi-head self-attention with RoPE on 8 Trainium2 NeuronCores.

Sharding: core = (batch b, head-group g) with b = core//2, g = core%2.
Each core computes QKV projections for its batch element restricted to its
8 heads (512 of 1024 projection rows), RoPE, causal attention, and the
partial output projection y_g = attn_g @ wo[:, g*512:(g+1)*512].T.  The host
sums the two head-group partials per batch element.

v3 redesign (vs the 466us v2 baseline):
v2's trace showed the PE at half clock (HAM K=4/8) for ~270us: phase 2's
per-unit PE work (~0.5us) sat under the ACT exp rate (~0.7us/unit), the
micro-idles kept re-throttling the PE, and cold matmuls (427ns vs 213ns)
then became the critical path.  v3 merges the phases: the projection
pipeline for s-blocks 4..15 is decomposed into small closures that are
pumped between attention score/PV matmuls, so the PE always has dense
independent work while ACT exps drain.  Engine totals land ~balanced
(PE ~215us, ACT ~210us, DVE ~200us).
- scores for the head pair (2hp, 2hp+1) sit in row groups 0-1 / 2-3
  (contraction 64 at base partitions 0/64) and execute concurrently.
- PV accumulates both heads of a pair into one 2-bank PSUM tile with
  per-bank start/stop flags; pv matmuls lag scores by 2 key-blocks.
- PSUM: proj 2 banks, scores 2, {transpose-chain, oproj} shared 2, PV 2.
"""
import math
import os
from contextlib import ExitStack

import numpy as np
import ml_dtypes

import concourse.bass as bass
import concourse.tile as tile
from concourse import bacc, mybir
from concourse import masks
from concourse.bass_utils import run_bass_kernel_spmd

F32 = mybir.dt.float32
BF16 = mybir.dt.bfloat16
EXPF = mybir.ActivationFunctionType.Exp

D = 1024          # d_model
NH = 16           # heads total
DK = 64           # head dim
S = 2048          # sequence
B = 4             # batch
THETA = 10000.0
HPG = 8           # heads per group (2 groups over 8 cores with 4 batches)
W = HPG * DK      # 512: local projection width
NSB = S // 128    # 16 s-blocks
NQG = 4           # 512-wide q groups
ESH = -3.0        # exp shift: exp(s/8 - 3); cancels in normalization
SCL = 1.0 / math.sqrt(DK)

MODE = "v3-merged (bf16)"
TRACE = bool(int(os.environ.get("KTRACE", "0")))

_cache = {}


def build_nc():
    nc = bacc.Bacc(None, target_bir_lowering=False, debug=False)

    xt = nc.dram_tensor("xt", [D, S], BF16, kind="ExternalInput")
    wqt = nc.dram_tensor("wqt", [D, W], BF16, kind="ExternalInput")
    wkt = nc.dram_tensor("wkt", [D, W], BF16, kind="ExternalInput")
    wvt = nc.dram_tensor("wvt", [D, W], BF16, kind="ExternalInput")
    wot = nc.dram_tensor("wot", [W, D], BF16, kind="ExternalInput")
    cosb = nc.dram_tensor("cosb", [S, W // 2], F32, kind="ExternalInput")
    sinb = nc.dram_tensor("sinb", [S, W // 2], F32, kind="ExternalInput")
    yp = nc.dram_tensor("yp", [S, D], F32, kind="ExternalOutput")

    xt3 = xt[:].rearrange("(jo p) s -> p jo s", p=128)       # [128, 8, S]
    wqt3 = wqt[:].rearrange("(jo p) i -> p jo i", p=128)     # [128, 8, W]
    wkt3 = wkt[:].rearrange("(jo p) i -> p jo i", p=128)
    wvt3 = wvt[:].rearrange("(jo p) i -> p jo i", p=128)
    wot3 = wot[:].rearrange("(jo p) i -> p jo i", p=128)     # [128, 4, D]

    with tile.TileContext(nc, pool_alloc_mode="queue") as tc, \
            ExitStack() as ctx:
        persist = ctx.enter_context(tc.tile_pool(name="persist", bufs=1))
        identb = persist.tile([128, 128], BF16, name="identb")
        masks.make_identity(nc, identb)
        bsh = persist.tile([128, 1], F32, name="bsh")
        nc.gpsimd.memset(bsh, ESH)

        # persistent activations: q^T / k^T as [128, slab, S] (slab j holds
        # the 128 dims of heads (2j, 2j+1)); v s-major in s-block PAIRS with
        # a ones column per head for the softmax denominator.
        qT4 = persist.tile([128, 4, S], BF16, name="qT4")
        kT4 = persist.tile([128, 4, S], BF16, name="kT4")
        vtp = [persist.tile([128, 2, HPG, DK + 1], BF16, name=f"vtp{i}")
               for i in range(NSB // 2)]

        wp = ctx.enter_context(tc.tile_pool(name="wp", bufs=1))
        p1t = ctx.enter_context(tc.tile_pool(name="p1t", bufs=3))
        p2t = ctx.enter_context(tc.tile_pool(name="p2t", bufs=3))
        aqp = ctx.enter_context(tc.tile_pool(name="aqp", bufs=2))
        expp = ctx.enter_context(tc.tile_pool(name="exp", bufs=6))
        ppp = ctx.enter_context(tc.tile_pool(name="ppp", bufs=2,
                                             space="PSUM"))
        scp = ctx.enter_context(tc.tile_pool(name="scp", bufs=2,
                                             space="PSUM"))
        mxp = ctx.enter_context(tc.tile_pool(name="mxp", bufs=2,
                                             space="PSUM"))
        pvp = ctx.enter_context(tc.tile_pool(name="pvp", bufs=1,
                                             space="PSUM"))

        # ---- weights: wq as 8 per-chunk tiles on the scalar queue so the
        # first projection matmul waits on 128KB, not 1MB; k/v/o on gpsimd
        wq_j = [wp.tile([128, W], BF16, name=f"wq_j{jo}") for jo in range(8)]
        for jo in range(8):
            nc.scalar.dma_start(wq_j[jo][:], wqt3[:, jo, :])
        wk_s = wp.tile([128, 8, W], BF16, name="wk_s")
        wv_s = wp.tile([128, 8, W], BF16, name="wv_s")
        wo_s = wp.tile([128, 4, D], BF16, name="wo_s")
        nc.gpsimd.dma_start(wk_s[:], wkt3[:])
        nc.gpsimd.dma_start(wv_s[:], wvt3[:])
        nc.gpsimd.dma_start(wo_s[:], wot3[:])

        def rope(ps, outt, c3, s3):
            # ps: [128, W] PSUM (pre-RoPE proj, s-major, heads as
            # [evens(32) | odds(32)] blocks); outt: [128, W] SBUF bf16
            pe = ps.rearrange("p (h eo c) -> p h eo c", eo=2, c=32)
            ein, oin = pe[:, :, 0, :], pe[:, :, 1, :]
            oe = outt.rearrange("p (h eo c) -> p h eo c", eo=2, c=32)
            eout, oout = oe[:, :, 0, :], oe[:, :, 1, :]
            ra = p1t.tile([128, 8, 32], F32, name="ra", tag="ra")
            rb = p1t.tile([128, 8, 32], F32, name="rb", tag="rb")
            nc.vector.tensor_mul(ra, ein, c3)
            nc.vector.tensor_mul(rb, oin, s3)
            nc.vector.tensor_sub(eout, ra, rb)
            rc = p1t.tile([128, 8, 32], F32, name="rc", tag="rc")
            rd = p1t.tile([128, 8, 32], F32, name="rd", tag="rd")
            nc.vector.tensor_mul(rc, ein, s3)
            nc.vector.tensor_mul(rd, oin, c3)
            nc.vector.tensor_add(oout, rc, rd)

        def proj_closures(sb):
            """Projection pipeline for s-block sb, as a list of small
            closures (each ~0.5-1us of one engine) pumped between
            attention matmuls."""
            s0 = sb * 128
            st = {}

            def load():
                xs = p1t.tile([128, 8, 128], BF16, name="xs", tag="xs")
                nc.sync.dma_start(xs[:], xt3[:, :, s0:s0 + 128])
                cs = p1t.tile([128, W // 2], F32, name="cs", tag="cs")
                nc.sync.dma_start(cs[:], cosb[s0:s0 + 128, :])
                sn = p1t.tile([128, W // 2], F32, name="sn", tag="sn")
                nc.sync.dma_start(sn[:], sinb[s0:s0 + 128, :])
                st["xs"], st["cs"], st["sn"] = xs, cs, sn

            def mk_mm(key, wsrc, lo, hi):
                def mm():
                    if key not in st:
                        st[key] = ppp.tile([128, W], F32, name=key, tag="pp")
                    dst, xs = st[key], st["xs"]
                    for jo in range(lo, hi):
                        if isinstance(wsrc, list):
                            w = wsrc[jo][:]
                        else:
                            w = wsrc[:, jo, :]
                        nc.tensor.matmul(dst[:], xs[:, jo, :], w,
                                         start=(jo == 0), stop=(jo == 7))
                return mm

            def ropeq():
                q_ro = p1t.tile([128, W], BF16, name="q_ro", tag="qro",
                                bufs=2)
                c3 = st["cs"].rearrange("p (h c) -> p h c", c=32)
                s3 = st["sn"].rearrange("p (h c) -> p h c", c=32)
                rope(st["pq"], q_ro, c3, s3)
                st["q_ro"] = q_ro

            def ropek():
                k_ro = p1t.tile([128, W], BF16, name="k_ro", tag="kro",
                                bufs=2)
                c3 = st["cs"].rearrange("p (h c) -> p h c", c=32)
                s3 = st["sn"].rearrange("p (h c) -> p h c", c=32)
                rope(st["pk"], k_ro, c3, s3)
                st["k_ro"] = k_ro

            def vev():
                # v eviction (+ ones column per head) on DVE
                v3 = vtp[sb // 2][:, sb % 2, :, :]       # [128, 8, 65]
                nc.vector.tensor_copy(
                    v3[:, :, 0:DK],
                    st["pv"].rearrange("p (h c) -> p h c", c=DK))
                nc.gpsimd.memset(v3[:, :, DK:DK + 1].bitcast(mybir.dt.uint16),
                                 0x3F80)

            def trans():
                # 8 chained bf16 transposes into one PSUM bank, then two
                # strided DVE evictions into the d-major slabs
                ptr8 = mxp.tile([128, 8, 128], BF16, name="ptr8", tag="mx")
                q_ro, k_ro = st["q_ro"], st["k_ro"]
                for pr in range(4):
                    nc.tensor.matmul(
                        ptr8[:, pr, :], q_ro[:, pr * 128:(pr + 1) * 128],
                        identb[:], is_transpose=True,
                        start=(pr == 0), stop=False)
                for pr in range(4):
                    nc.tensor.matmul(
                        ptr8[:, 4 + pr, :], k_ro[:, pr * 128:(pr + 1) * 128],
                        identb[:], is_transpose=True,
                        start=False, stop=(pr == 3))
                nc.vector.tensor_copy(qT4[:, :, s0:s0 + 128], ptr8[:, 0:4, :])
                nc.vector.tensor_copy(kT4[:, :, s0:s0 + 128], ptr8[:, 4:8, :])

            return [load,
                    mk_mm("pq", wq_j, 0, 4), mk_mm("pq", wq_j, 4, 8),
                    ropeq,
                    mk_mm("pk", wk_s, 0, 4), mk_mm("pk", wk_s, 4, 8),
                    ropek,
                    mk_mm("pv", wv_s, 0, 4), mk_mm("pv", wv_s, 4, 8),
                    vev, trans]

        # prologue: s-blocks 0-3 run eagerly; 4-15 queued as filler
        for sb in range(4):
            for cl in proj_closures(sb):
                cl()
        filler = []
        for sb in range(4, NSB):
            for cl in proj_closures(sb):
                filler.append((sb, cl))
        fpos = [0]

        def pump(n=1):
            hi = min(fpos[0] + n, len(filler))
            for i in range(fpos[0], hi):
                filler[i][1]()
            fpos[0] = hi

        def flush_blocks(upto):
            while fpos[0] < len(filler) and filler[fpos[0]][0] <= upto:
                filler[fpos[0]][1]()
                fpos[0] += 1

        # ------------------- attention + output projection ----------------
        def attn_pair(qg, hp, stage8, aqb):
            """Scores + exp + PV for heads (2hp, 2hp+1): scores run
            concurrently in row groups 0-1/2-3; PV accumulates into a
            2-bank pair tile, lagging scores by 2 key-blocks."""
            q0 = qg * 512
            pvh = pvp.tile([DK + 1, 2, 512], F32, name="pvh", tag="pv")
            emitters = []

            def drive(last=False):
                n = len(emitters)
                hi = n if last else max(n - 4, 0)
                for i in range(drive.done, hi):
                    emitters[i](i < 2, last and i >= n - 2)
                drive.done = hi
            drive.done = 0

            nkb = 4 * qg + 4
            for kb in range(nkb):        # one unit per 128-key block
                off = kb - 4 * qg
                c0 = 128 * max(off, 0)
                wd = 512 - c0
                for hh in range(2):
                    r0 = 64 * hh
                    h = 2 * hp + hh
                    sc = scp.tile([128, 512], F32, name="sc", tag="sc")
                    nc.tensor.matmul(
                        sc[:, c0:512],
                        kT4[r0:r0 + DK, hp, kb * 128:(kb + 1) * 128],
                        qT4[r0:r0 + DK, hp, q0 + c0:q0 + 512],
                        start=True, stop=True)
                    ex = expp.tile([128, 512], BF16, name="ex", tag="ex")
                    nc.scalar.activation(ex[:, 0:wd], sc[:, c0:512],
                                         EXPF, scale=SCL, bias=bsh[:, 0:1])
                    if off >= 0:
                        # causal mask for the 128-wide diagonal square on
                        # the (idle) gpsimd: zero the q < k half post-exp
                        nc.gpsimd.affine_select(
                            out=ex[:, 0:128], in_=ex[:, 0:128],
                            compare_op=mybir.AluOpType.is_ge, fill=0.0,
                            base=0, pattern=[[1, 128]],
                            channel_multiplier=-1)
                    emitters.append(
                        lambda stf, spf, kb=kb, hh=hh, h=h, c0=c0, wd=wd,
                        ex=ex:
                        nc.tensor.matmul(
                            pvh[:, hh, c0:512],
                            vtp[kb // 2][:, kb % 2, h, :],
                            ex[:, 0:wd], start=stf, stop=spf))
                    drive()
                    pump(1)
            drive(last=True)

            # evict unnormalized attn^T rows; stage the denominator rows
            for hh in range(2):
                nc.vector.tensor_copy(aqb[64 * hh:64 * hh + 64, hp, :],
                                      pvh[0:DK, hh, :])
                dsb = p2t.tile([1, 512], F32, name="dsb", tag="dsb",
                               bufs=3)
                nc.vector.tensor_copy(dsb[:], pvh[DK:DK + 1, hh, :])
                nc.sync.dma_start(stage8[2 * hp + hh:2 * hp + hh + 1, :],
                                  dsb[:])

        def make_deferred(qg, stage8, aqb):
            """Normalize + output projection for q-group qg, split into
            9 steps that are woven into the next q-group's pair loop."""
            steps = []
            rall = [None]

            def s_recip():
                r = p2t.tile([HPG, 512], F32, name="rall8", tag="r8",
                             bufs=2)
                nc.vector.reciprocal(r[:], stage8[:])
                rall[0] = r
            steps.append(s_recip)

            def norm_pair(hp):
                for h in (2 * hp, 2 * hp + 1):
                    slab, r0 = h // 2, 64 * (h % 2)
                    rsb = p2t.tile([1, 512], F32, name="rsb", tag="rsb",
                                   bufs=2)
                    nc.sync.dma_start(rsb[:], rall[0][h:h + 1, :])
                    rbc = p2t.tile([128, 512], F32, name="rbc",
                                   tag="rbc", bufs=2)
                    nc.gpsimd.partition_broadcast(rbc[:], rsb[:],
                                                  channels=128)
                    nc.vector.tensor_mul(aqb[r0:r0 + 64, slab, :],
                                         aqb[r0:r0 + 64, slab, :],
                                         rbc[r0:r0 + 64, :])
            for hp in range(4):
                steps.append(lambda hp=hp: norm_pair(hp))

            def oproj(sbl):
                s0 = qg * 512 + sbl * 128
                yt = p2t.tile([128, 1024], F32, name="yt", tag="yt",
                              bufs=2)
                for ih in range(2):
                    # 1-bank py double-buffered via the shared mx pool:
                    # the eviction of half ih overlaps the next half
                    py = mxp.tile([128, 512], F32, name="py", tag="mx")
                    for j in range(4):
                        nc.tensor.matmul(
                            py[:],
                            aqb[:, j, sbl * 128:(sbl + 1) * 128],
                            wo_s[:, j, ih * 512:(ih + 1) * 512],
                            start=(j == 0), stop=(j == 3))
                    nc.vector.tensor_copy(
                        yt[:, ih * 512:(ih + 1) * 512], py[:])
                nc.sync.dma_start(yp[s0:s0 + 128, :], yt[:])
            for sbl in range(4):
                steps.append(lambda sbl=sbl: oproj(sbl))
            return steps

        deferred = []
        for qg in range(NQG):
            flush_blocks(4 * qg + 3)
            stage8 = p2t.tile([HPG, 512], F32, name="stage8",
                              tag="stage8", bufs=2)
            aqb = aqp.tile([128, 4, 512], BF16, name="aqb", tag="aqb",
                           bufs=2)
            for hp in range(4):
                attn_pair(qg, hp, stage8, aqb)
                for stp in deferred[2 * hp:2 * hp + 2]:
                    stp()
            for stp in deferred[8:]:
                stp()
            deferred = make_deferred(qg, stage8, aqb)
        for stp in deferred:
            stp()

    nc.compile()
    return nc


def _prep_inputs(x, token_positions, wq, wk, wv, wo):
    bf16 = ml_dtypes.bfloat16
    # per-head permutation: [0,2,...,62, 1,3,...,63] (evens then odds)
    pi = np.concatenate([np.arange(0, DK, 2), np.arange(1, DK, 2)])
    perm = (np.arange(NH)[:, None] * DK + pi[None, :]).reshape(-1)
    wq_p = wq[perm, :]
    wk_p = wk[perm, :]

    pos = np.asarray(token_positions).astype(np.float32)
    thetas = (1.0 / (THETA ** (2.0 * np.arange(DK // 2, dtype=np.float32)
                               / DK))).astype(np.float32)
    ang = np.outer(pos, thetas).astype(np.float32)          # [S, 32]
    cos = np.tile(np.cos(ang), (1, HPG)).astype(np.float32)  # [S, 256]
    sin = np.tile(np.sin(ang), (1, HPG)).astype(np.float32)

    in_maps = []
    for core in range(8):
        b, g = core // 2, core % 2
        gs = slice(g * W, (g + 1) * W)
        in_maps.append({
            "xt": np.ascontiguousarray(x[b].T).astype(bf16),
            "wqt": np.ascontiguousarray(wq_p[gs, :].T).astype(bf16),
            "wkt": np.ascontiguousarray(wk_p[gs, :].T).astype(bf16),
            "wvt": np.ascontiguousarray(wv[gs, :].T).astype(bf16),
            "wot": np.ascontiguousarray(wo[:, gs].T).astype(bf16),
            "cosb": cos,
            "sinb": sin,
        })
    return in_maps


last_exec_time_ns = None


def _install_ntff_hook_shim():
    """This image's antenv lacks axon_hooks; wire the ctypes NTFF hook from
    trn_agent_boot so trace=True yields HW exec times."""
    import sys as _sys
    import types as _types
    try:
        from antenv import axon_hooks  # noqa: F401
        return
    except ImportError:
        pass
    from trn_agent_boot.trn_boot import _ntff_profile_via_ctypes
    hook = _ntff_profile_via_ctypes("/opt/axon/libaxon_pjrt.so")
    mod = _types.ModuleType("antenv.axon_hooks")
    mod.get_axon_ntff_profile_hook = lambda: hook
    _sys.modules["antenv.axon_hooks"] = mod


def kernel(x, token_positions, wq, wk, wv, wo):
    global last_exec_time_ns
    x = np.asarray(x, dtype=np.float32)
    token_positions = np.asarray(token_positions)
    wq = np.asarray(wq, dtype=np.float32)
    wk = np.asarray(wk, dtype=np.float32)
    wv = np.asarray(wv, dtype=np.float32)
    wo = np.asarray(wo, dtype=np.float32)

    if "nc" not in _cache:
        _cache["nc"] = build_nc()
    nc = _cache["nc"]

    in_maps = _prep_inputs(x, token_positions, wq, wk, wv, wo)
    res = None
    if TRACE:
        try:
            _install_ntff_hook_shim()
            res = run_bass_kernel_spmd(nc, in_maps, list(range(8)),
                                       trace=True,
                                       trace_cores=list(range(8)))
        except Exception as e:  # profiling must never sink correctness
            print(f"trace run failed ({type(e).__name__}: {e}); "
                  f"retrying untraced")
            res = None
    if res is None:
        res = run_bass_kernel_spmd(nc, in_maps, list(range(8)))
    last_exec_time_ns = res.exec_time_ns

    out = np.empty((B, S, D), dtype=np.float32)
    for b in range(B):
        out[b] = res.results[2 * b]["yp"] + res.results[2 * b + 1]["yp"]
    return out


# revision 8
# speedup vs baseline: 1.1148x; 1.0355x over previous
"""Causal multi-head self-attention with RoPE on 8 Trainium2 NeuronCores.

Sharding: core = (batch b, head-group g) with b = core//2, g = core%2.
Each core computes QKV projections for its batch element restricted to its
8 heads (512 of 1024 projection rows), RoPE, causal attention, and the
partial output projection y_g = attn_g @ wo[:, g*512:(g+1)*512].T.  The host
sums the two head-group partials per batch element.

v3 redesign (vs the 466us v2 baseline):
v2's trace showed the PE at half clock (HAM K=4/8) for ~270us: phase 2's
per-unit PE work (~0.5us) sat under the ACT exp rate (~0.7us/unit), the
micro-idles kept re-throttling the PE, and cold matmuls (427ns vs 213ns)
then became the critical path.  v3 merges the phases: the projection
pipeline for s-blocks 4..15 is decomposed into small closures that are
pumped between attention score/PV matmuls, so the PE always has dense
independent work while ACT exps drain.  Engine totals land ~balanced
(PE ~215us, ACT ~210us, DVE ~200us).
- scores for the head pair (2hp, 2hp+1) sit in row groups 0-1 / 2-3
  (contraction 64 at base partitions 0/64) and execute concurrently.
- PV accumulates both heads of a pair into one 2-bank PSUM tile with
  per-bank start/stop flags; pv matmuls lag scores by 2 key-blocks.
- PSUM: proj 2 banks, scores 2, {transpose-chain, oproj} shared 2, PV 2.
"""
import math
import os
from contextlib import ExitStack

import numpy as np
import ml_dtypes

import concourse.bass as bass
import concourse.tile as tile
from concourse import bacc, mybir
from concourse import masks
from concourse.bass_utils import run_bass_kernel_spmd

F32 = mybir.dt.float32
BF16 = mybir.dt.bfloat16
EXPF = mybir.ActivationFunctionType.Exp

D = 1024          # d_model
NH = 16           # heads total
DK = 64           # head dim
S = 2048          # sequence
B = 4             # batch
THETA = 10000.0
HPG = 8           # heads per group (2 groups over 8 cores with 4 batches)
W = HPG * DK      # 512: local projection width
NSB = S // 128    # 16 s-blocks
NQG = 4           # 512-wide q groups
ESH = -3.0        # exp shift: exp(s/8 - 3); cancels in normalization
SCL = 1.0 / math.sqrt(DK)

MODE = "v4-pairexp (bf16)"
TRACE = bool(int(os.environ.get("KTRACE", "0")))

_cache = {}


def build_nc():
    nc = bacc.Bacc(None, target_bir_lowering=False, debug=False)

    xt = nc.dram_tensor("xt", [D, S], BF16, kind="ExternalInput")
    wqt = nc.dram_tensor("wqt", [D, W], BF16, kind="ExternalInput")
    wkt = nc.dram_tensor("wkt", [D, W], BF16, kind="ExternalInput")
    wvt = nc.dram_tensor("wvt", [D, W], BF16, kind="ExternalInput")
    wot = nc.dram_tensor("wot", [W, D], BF16, kind="ExternalInput")
    cosb = nc.dram_tensor("cosb", [S, W // 2], F32, kind="ExternalInput")
    sinb = nc.dram_tensor("sinb", [S, W // 2], F32, kind="ExternalInput")
    yp = nc.dram_tensor("yp", [S, D], F32, kind="ExternalOutput")

    xt3 = xt[:].rearrange("(jo p) s -> p jo s", p=128)       # [128, 8, S]
    wqt3 = wqt[:].rearrange("(jo p) i -> p jo i", p=128)     # [128, 8, W]
    wkt3 = wkt[:].rearrange("(jo p) i -> p jo i", p=128)
    wvt3 = wvt[:].rearrange("(jo p) i -> p jo i", p=128)
    wot3 = wot[:].rearrange("(jo p) i -> p jo i", p=128)     # [128, 4, D]

    with tile.TileContext(nc, pool_alloc_mode="queue") as tc, \
            ExitStack() as ctx:
        persist = ctx.enter_context(tc.tile_pool(name="persist", bufs=1))
        identb = persist.tile([128, 128], BF16, name="identb")
        masks.make_identity(nc, identb)
        bsh = persist.tile([128, 1], F32, name="bsh")
        nc.gpsimd.memset(bsh, ESH)

        # persistent activations: q^T / k^T as [128, slab, S] (slab j holds
        # the 128 dims of heads (2j, 2j+1)); v s-major in s-block PAIRS with
        # a ones column per head for the softmax denominator.
        qT4 = persist.tile([128, 4, S], BF16, name="qT4")
        kT4 = persist.tile([128, 4, S], BF16, name="kT4")
        vtp = [persist.tile([128, 2, HPG, DK + 1], BF16, name=f"vtp{i}")
               for i in range(NSB // 2)]

        wp = ctx.enter_context(tc.tile_pool(name="wp", bufs=1))
        p1t = ctx.enter_context(tc.tile_pool(name="p1t", bufs=3))
        p2t = ctx.enter_context(tc.tile_pool(name="p2t", bufs=3))
        aqp = ctx.enter_context(tc.tile_pool(name="aqp", bufs=2))
        expp = ctx.enter_context(tc.tile_pool(name="exp", bufs=6))
        # PSUM: proj 2 banks; {score-pairs, transpose-chain, oproj-pair}
        # share one 2-slot pool of 2-bank slots; PV pair 2 banks.
        ppp = ctx.enter_context(tc.tile_pool(name="ppp", bufs=2,
                                             space="PSUM"))
        scp = ctx.enter_context(tc.tile_pool(name="scp", bufs=2,
                                             space="PSUM"))
        pvp = ctx.enter_context(tc.tile_pool(name="pvp", bufs=1,
                                             space="PSUM"))

        # ---- weights: wq as 8 per-chunk tiles on the scalar queue so the
        # first projection matmul waits on 128KB, not 1MB; k/v/o on gpsimd
        wq_j = [wp.tile([128, W], BF16, name=f"wq_j{jo}") for jo in range(8)]
        for jo in range(8):
            nc.scalar.dma_start(wq_j[jo][:], wqt3[:, jo, :])
        wk_s = wp.tile([128, 8, W], BF16, name="wk_s")
        wv_s = wp.tile([128, 8, W], BF16, name="wv_s")
        wo_s = wp.tile([128, 4, D], BF16, name="wo_s")
        nc.gpsimd.dma_start(wk_s[:], wkt3[:])
        nc.gpsimd.dma_start(wv_s[:], wvt3[:])
        nc.gpsimd.dma_start(wo_s[:], wot3[:])

        def rope(ps, outt, c3, s3):
            # ps: [128, W] PSUM (pre-RoPE proj, s-major, heads as
            # [evens(32) | odds(32)] blocks); outt: [128, W] SBUF bf16
            pe = ps.rearrange("p (h eo c) -> p h eo c", eo=2, c=32)
            ein, oin = pe[:, :, 0, :], pe[:, :, 1, :]
            oe = outt.rearrange("p (h eo c) -> p h eo c", eo=2, c=32)
            eout, oout = oe[:, :, 0, :], oe[:, :, 1, :]
            ra = p1t.tile([128, 8, 32], F32, name="ra", tag="ra")
            rb = p1t.tile([128, 8, 32], F32, name="rb", tag="rb")
            nc.vector.tensor_mul(ra, ein, c3)
            nc.vector.tensor_mul(rb, oin, s3)
            nc.vector.tensor_sub(eout, ra, rb)
            rc = p1t.tile([128, 8, 32], F32, name="rc", tag="rc")
            rd = p1t.tile([128, 8, 32], F32, name="rd", tag="rd")
            nc.vector.tensor_mul(rc, ein, s3)
            nc.vector.tensor_mul(rd, oin, c3)
            nc.vector.tensor_add(oout, rc, rd)

        def proj_closures(sb):
            """Projection pipeline for s-block sb, as a list of small
            closures (each ~0.5-1us of one engine) pumped between
            attention matmuls."""
            s0 = sb * 128
            st = {}

            def load():
                xs = p1t.tile([128, 8, 128], BF16, name="xs", tag="xs")
                nc.sync.dma_start(xs[:], xt3[:, :, s0:s0 + 128])
                cs = p1t.tile([128, W // 2], F32, name="cs", tag="cs")
                nc.sync.dma_start(cs[:], cosb[s0:s0 + 128, :])
                sn = p1t.tile([128, W // 2], F32, name="sn", tag="sn")
                nc.sync.dma_start(sn[:], sinb[s0:s0 + 128, :])
                st["xs"], st["cs"], st["sn"] = xs, cs, sn

            def mk_mm(key, wsrc, lo, hi):
                def mm():
                    if key not in st:
                        st[key] = ppp.tile([128, W], F32, name=key, tag="pp")
                    dst, xs = st[key], st["xs"]
                    for jo in range(lo, hi):
                        if isinstance(wsrc, list):
                            w = wsrc[jo][:]
                        else:
                            w = wsrc[:, jo, :]
                        nc.tensor.matmul(dst[:], xs[:, jo, :], w,
                                         start=(jo == 0), stop=(jo == 7))
                return mm

            def ropeq():
                q_ro = p1t.tile([128, W], BF16, name="q_ro", tag="qro",
                                bufs=2)
                c3 = st["cs"].rearrange("p (h c) -> p h c", c=32)
                s3 = st["sn"].rearrange("p (h c) -> p h c", c=32)
                rope(st["pq"], q_ro, c3, s3)
                st["q_ro"] = q_ro

            def ropek():
                k_ro = p1t.tile([128, W], BF16, name="k_ro", tag="kro",
                                bufs=2)
                c3 = st["cs"].rearrange("p (h c) -> p h c", c=32)
                s3 = st["sn"].rearrange("p (h c) -> p h c", c=32)
                rope(st["pk"], k_ro, c3, s3)
                st["k_ro"] = k_ro

            def vev():
                # v eviction (+ ones column per head) on DVE
                v3 = vtp[sb // 2][:, sb % 2, :, :]       # [128, 8, 65]
                nc.vector.tensor_copy(
                    v3[:, :, 0:DK],
                    st["pv"].rearrange("p (h c) -> p h c", c=DK))
                nc.gpsimd.memset(v3[:, :, DK:DK + 1].bitcast(mybir.dt.uint16),
                                 0x3F80)

            def trans():
                # 8 chained bf16 transposes into one PSUM bank, then two
                # strided DVE evictions into the d-major slabs
                ptr8 = scp.tile([128, 8, 128], BF16, name="ptr8", tag="sc")
                q_ro, k_ro = st["q_ro"], st["k_ro"]
                for pr in range(4):
                    nc.tensor.matmul(
                        ptr8[:, pr, :], q_ro[:, pr * 128:(pr + 1) * 128],
                        identb[:], is_transpose=True,
                        start=(pr == 0), stop=False)
                for pr in range(4):
                    nc.tensor.matmul(
                        ptr8[:, 4 + pr, :], k_ro[:, pr * 128:(pr + 1) * 128],
                        identb[:], is_transpose=True,
                        start=False, stop=(pr == 3))
                nc.vector.tensor_copy(qT4[:, :, s0:s0 + 128], ptr8[:, 0:4, :])
                nc.vector.tensor_copy(kT4[:, :, s0:s0 + 128], ptr8[:, 4:8, :])

            return [load,
                    mk_mm("pq", wq_j, 0, 4), mk_mm("pq", wq_j, 4, 8),
                    ropeq,
                    mk_mm("pk", wk_s, 0, 4), mk_mm("pk", wk_s, 4, 8),
                    ropek,
                    mk_mm("pv", wv_s, 0, 4), mk_mm("pv", wv_s, 4, 8),
                    vev, trans]

        # absorb the ~2.7us exp table load off the critical path
        wdum = p2t.tile([1, 1], F32, name="wdum", tag="wdum")
        nc.scalar.activation(wdum[:], bsh[0:1, 0:1], EXPF)

        # prologue: s-blocks 0-3 run eagerly (loads first so the sync
        # queue streams x while the PE chews); 4-15 queued as filler
        pro = [proj_closures(sb) for sb in range(4)]
        for cls in pro:
            cls[0]()          # loads
        for cls in pro:
            for cl in cls[1:]:
                cl()
        filler = []
        for sb in range(4, NSB):
            for cl in proj_closures(sb):
                filler.append((sb, cl))
        fpos = [0]

        def pump(n=1):
            hi = min(fpos[0] + n, len(filler))
            for i in range(fpos[0], hi):
                filler[i][1]()
            fpos[0] = hi

        def flush_blocks(upto):
            while fpos[0] < len(filler) and filler[fpos[0]][0] <= upto:
                filler[fpos[0]][1]()
                fpos[0] += 1

        # ------------------- attention + output projection ----------------
        def attn_pair(qg, hp, stage8, aqb):
            """Scores + exp + PV for heads (2hp, 2hp+1): scores run
            concurrently in row groups 0-1/2-3; PV accumulates into a
            2-bank pair tile, lagging scores by 2 key-blocks."""
            q0 = qg * 512
            pvh = pvp.tile([DK + 1, 2, 512], F32, name="pvh", tag="pv")
            emitters = []

            def drive(last=False):
                n = len(emitters)
                hi = n if last else max(n - 4, 0)
                for i in range(drive.done, hi):
                    emitters[i](i < 2, last and i >= n - 2)
                drive.done = hi
            drive.done = 0

            nkb = 4 * qg + 4
            for kb in range(nkb):        # one unit per 128-key block
                off = kb - 4 * qg
                c0 = 128 * max(off, 0)
                wd = 512 - c0
                sc = scp.tile([128, 2, 512], F32, name="sc", tag="sc")
                for hh in range(2):
                    nc.tensor.matmul(
                        sc[:, hh, c0:512],
                        kT4[64 * hh:64 * hh + DK, hp,
                            kb * 128:(kb + 1) * 128],
                        qT4[64 * hh:64 * hh + DK, hp, q0 + c0:q0 + 512],
                        start=True, stop=True)
                # one pair-wide exp (both heads, 2 PSUM banks) halves the
                # per-ACTIVATE fixed overhead
                ex = expp.tile([128, 2, 512], BF16, name="ex", tag="ex",
                               bufs=4)
                nc.scalar.activation(ex[:, :, 0:wd], sc[:, :, c0:512],
                                     EXPF, scale=SCL, bias=bsh[:, 0:1])
                if off >= 0:
                    # causal mask for the 128-wide diagonal square on
                    # the (idle) gpsimd: zero the q < k half post-exp
                    for hh in range(2):
                        nc.gpsimd.affine_select(
                            out=ex[:, hh, 0:128], in_=ex[:, hh, 0:128],
                            compare_op=mybir.AluOpType.is_ge, fill=0.0,
                            base=0, pattern=[[1, 128]],
                            channel_multiplier=-1)
                for hh in range(2):
                    emitters.append(
                        lambda stf, spf, kb=kb, hh=hh, c0=c0, wd=wd,
                        ex=ex:
                        nc.tensor.matmul(
                            pvh[:, hh, c0:512],
                            vtp[kb // 2][:, kb % 2, 2 * hp + hh, :],
                            ex[:, hh, 0:wd], start=stf, stop=spf))
                    drive()
                pump(1)
            drive(last=True)

            # evict unnormalized attn^T rows; stage the denominator rows
            for hh in range(2):
                nc.vector.tensor_copy(aqb[64 * hh:64 * hh + 64, hp, :],
                                      pvh[0:DK, hh, :])
                dsb = p2t.tile([1, 512], F32, name="dsb", tag="dsb",
                               bufs=3)
                nc.vector.tensor_copy(dsb[:], pvh[DK:DK + 1, hh, :])
                nc.sync.dma_start(stage8[2 * hp + hh:2 * hp + hh + 1, :],
                                  dsb[:])

        def make_deferred(qg, stage8, aqb):
            """Normalize + output projection for q-group qg, split into
            9 steps that are woven into the next q-group's pair loop."""
            steps = []
            rall = [None]

            def s_recip():
                r = p2t.tile([HPG, 512], F32, name="rall8", tag="r8",
                             bufs=2)
                nc.vector.reciprocal(r[:], stage8[:])
                rall[0] = r
            steps.append(s_recip)

            def norm_pair(hp):
                for h in (2 * hp, 2 * hp + 1):
                    slab, r0 = h // 2, 64 * (h % 2)
                    rsb = p2t.tile([1, 512], F32, name="rsb", tag="rsb",
                                   bufs=2)
                    nc.sync.dma_start(rsb[:], rall[0][h:h + 1, :])
                    rbc = p2t.tile([128, 512], F32, name="rbc",
                                   tag="rbc", bufs=2)
                    nc.gpsimd.partition_broadcast(rbc[:], rsb[:],
                                                  channels=128)
                    nc.vector.tensor_mul(aqb[r0:r0 + 64, slab, :],
                                         aqb[r0:r0 + 64, slab, :],
                                         rbc[r0:r0 + 64, :])
            for hp in range(4):
                steps.append(lambda hp=hp: norm_pair(hp))

            def oproj(sbl):
                s0 = qg * 512 + sbl * 128
                yt = p2t.tile([128, 1024], F32, name="yt", tag="yt",
                              bufs=2)
                py = scp.tile([128, 2, 512], F32, name="py", tag="sc")
                for ih in range(2):
                    for j in range(4):
                        nc.tensor.matmul(
                            py[:, ih, :],
                            aqb[:, j, sbl * 128:(sbl + 1) * 128],
                            wo_s[:, j, ih * 512:(ih + 1) * 512],
                            start=(j == 0), stop=(j == 3))
                nc.vector.tensor_copy(yt[:], py[:])
                nc.sync.dma_start(yp[s0:s0 + 128, :], yt[:])
            for sbl in range(4):
                steps.append(lambda sbl=sbl: oproj(sbl))
            return steps

        deferred = []
        for qg in range(NQG):
            flush_blocks(4 * qg + 3)
            stage8 = p2t.tile([HPG, 512], F32, name="stage8",
                              tag="stage8", bufs=2)
            aqb = aqp.tile([128, 4, 512], BF16, name="aqb", tag="aqb",
                           bufs=2)
            for hp in range(4):
                attn_pair(qg, hp, stage8, aqb)
                for stp in deferred[2 * hp:2 * hp + 2]:
                    stp()
            for stp in deferred[8:]:
                stp()
            deferred = make_deferred(qg, stage8, aqb)
        for stp in deferred:
            stp()

    nc.compile()
    return nc


def _prep_inputs(x, token_positions, wq, wk, wv, wo):
    bf16 = ml_dtypes.bfloat16
    # per-head permutation: [0,2,...,62, 1,3,...,63] (evens then odds)
    pi = np.concatenate([np.arange(0, DK, 2), np.arange(1, DK, 2)])
    perm = (np.arange(NH)[:, None] * DK + pi[None, :]).reshape(-1)
    wq_p = wq[perm, :]
    wk_p = wk[perm, :]

    pos = np.asarray(token_positions).astype(np.float32)
    thetas = (1.0 / (THETA ** (2.0 * np.arange(DK // 2, dtype=np.float32)
                               / DK))).astype(np.float32)
    ang = np.outer(pos, thetas).astype(np.float32)          # [S, 32]
    cos = np.tile(np.cos(ang), (1, HPG)).astype(np.float32)  # [S, 256]
    sin = np.tile(np.sin(ang), (1, HPG)).astype(np.float32)

    in_maps = []
    for core in range(8):
        b, g = core // 2, core % 2
        gs = slice(g * W, (g + 1) * W)
        in_maps.append({
            "xt": np.ascontiguousarray(x[b].T).astype(bf16),
            "wqt": np.ascontiguousarray(wq_p[gs, :].T).astype(bf16),
            "wkt": np.ascontiguousarray(wk_p[gs, :].T).astype(bf16),
            "wvt": np.ascontiguousarray(wv[gs, :].T).astype(bf16),
            "wot": np.ascontiguousarray(wo[:, gs].T).astype(bf16),
            "cosb": cos,
            "sinb": sin,
        })
    return in_maps


last_exec_time_ns = None


def _install_ntff_hook_shim():
    """This image's antenv lacks axon_hooks; wire the ctypes NTFF hook from
    trn_agent_boot so trace=True yields HW exec times."""
    import sys as _sys
    import types as _types
    try:
        from antenv import axon_hooks  # noqa: F401
        return
    except ImportError:
        pass
    from trn_agent_boot.trn_boot import _ntff_profile_via_ctypes
    hook = _ntff_profile_via_ctypes("/opt/axon/libaxon_pjrt.so")
    mod = _types.ModuleType("antenv.axon_hooks")
    mod.get_axon_ntff_profile_hook = lambda: hook
    _sys.modules["antenv.axon_hooks"] = mod


def kernel(x, token_positions, wq, wk, wv, wo):
    global last_exec_time_ns
    x = np.asarray(x, dtype=np.float32)
    token_positions = np.asarray(token_positions)
    wq = np.asarray(wq, dtype=np.float32)
    wk = np.asarray(wk, dtype=np.float32)
    wv = np.asarray(wv, dtype=np.float32)
    wo = np.asarray(wo, dtype=np.float32)

    if "nc" not in _cache:
        _cache["nc"] = build_nc()
    nc = _cache["nc"]

    in_maps = _prep_inputs(x, token_positions, wq, wk, wv, wo)
    res = None
    if TRACE:
        try:
            _install_ntff_hook_shim()
            res = run_bass_kernel_spmd(nc, in_maps, list(range(8)),
                                       trace=True,
                                       trace_cores=list(range(8)))
        except Exception as e:  # profiling must never sink correctness
            print(f"trace run failed ({type(e).__name__}: {e}); "
                  f"retrying untraced")
            res = None
    if res is None:
        res = run_bass_kernel_spmd(nc, in_maps, list(range(8)))
    last_exec_time_ns = res.exec_time_ns

    out = np.empty((B, S, D), dtype=np.float32)
    for b in range(B):
        out[b] = res.results[2 * b]["yp"] + res.results[2 * b + 1]["yp"]
    return out
